# revision 1
# baseline (speedup 1.0000x reference)
"""GPT forward (6-layer, E=768, H=12, N=1024, B=2, V=50257) on 8 TRN2 cores.

Sharding: sequence-sharded layers (cores 0-3 batch 0, cores 4-7 batch 1;
core in-group index g owns row-blocks g and 7-g of its batch), one fused K/V
AllGather per layer within each 4-core group, one 8-core AllGather of the
final LN output, then a vocab-sharded lm_head (each core computes all 2048
rows x a 6283-wide vocab slice; host concatenates slices).

Compute: bf16 matmul inputs, fp32 PSUM/residual/LN. LN gammas are folded into
the following weights host-side, betas into biases. The causal mask is
multiplicative bf16 applied after exp (scores are small, so softmax's
max-subtraction is safely skipped); softmax denominators come from an
appended ones-column on V.
"""

from contextlib import ExitStack

import numpy as np
import ml_dtypes

import concourse.bass as bass
import concourse.mybir as mybir
import concourse.tile as tile
from concourse.bass_utils import run_bass_kernel_spmd
from concourse.vector_clock import ScopedClock, VectorClock

F32 = mybir.dt.float32
BF16 = mybir.dt.bfloat16
AF = mybir.ActivationFunctionType
BF = ml_dtypes.bfloat16
FP8 = mybir.dt.float8e4

V, E, N, H, L = 50257, 768, 1024, 12, 6
HD = E // H          # 64
FF = 4 * E           # 3072
B = 2
KT = E // 128        # 6 feature k-tiles
MT_QKV = 3 * KT      # 18 m-tiles for fused qkv
MT_FF = FF // 128    # 24
R = 256              # rows per core
NB = 8               # 128-row blocks per batch
VP = 6283            # vocab slice per core (8*6283 = 50264 >= V)
VPAD = 13 * 512      # host-side padded vocab slice (6656)
NCH = 13             # head vocab chunks of 512 (last used width = 139)
KV_GROUPS = [[0, 1, 2, 3], [4, 5, 6, 7]]
H8_GROUPS = [[0, 1, 2, 3, 4, 5, 6, 7]]
CC_KV_SZ = E * R + 2 * R * (H * 65)  # bytes: K as fp8, V as bf16
EPS = 1e-5


class _TileContext(tile.TileContext):
    """This image's walrus rejects Drain instructions with >1 sync-wait.
    Split the kernel-tail drain into one Drain per pending proc."""

    def _drain_and_barrier(self, tick_clock, wait_clock):
        nc = self.nc
        vec = tick_clock.global_clock
        n = len(vec)
        for proc in range(n):
            t = vec[proc]
            if t <= 0:
                continue
            sub = VectorClock([t if i == proc else 0 for i in range(n)])
            d = nc.sync.drain()
            wait_clock.add_sem_waits(d.ins, ScopedClock({None: sub}))
        nc.sync.drain()
        nc.all_engine_barrier()
        assert self.sems is not None
        popped = nc._tile_sem_poison_stack.pop()
        assert popped is self._sem_poison
        nc.clear_and_free_semaphores(list(self.sems.allocated().values()))
        nc.all_engine_barrier()


def _split_multi_waits(nc):
    """This walrus build encodes at most one sync-wait per instruction.
    Hoist extra waits onto NoOps inserted just before, on the same engine."""
    ctr = 0
    for bb in nc.main_func.blocks:
        il = bb.instructions
        out_l = []
        for ins in il:
            si = ins.sync_info
            if si is not None and si.on_wait is not None and len(si.on_wait) > 1:
                waits = list(si.on_wait)
                for w in waits[:-1]:
                    noop = mybir.InstNoOp(name=f"wsplit_{ctr}", ins=[], outs=[])
                    ctr += 1
                    noop.engine = ins.engine
                    noop.sync_info = type(si)(on_wait=[w], on_update=[])
                    out_l.append(noop)
                si.on_wait = waits[-1:]
            out_l.append(ins)
        il[:] = out_l


def _bcast_row(t, row, p=128):
    """AP reading DRAM row `t[row]` broadcast across p partitions."""
    base = t[row] if row is not None else t[:]
    return bass.AP(
        tensor=base.tensor, offset=base.offset,
        ap=[[0, p]] + [list(x) for x in base.ap])


def build_nc(use_bo, use_b2):
    nc = bass.Bass(num_devices=8)

    h0_in = nc.declare_dram_parameter("h0", [2, 128, E], F32, isOutput=False)
    # pre-tiled: [L, 18, 128, KT*128]  (m-tile, partition=feat%128, kt*128+mcol)
    wqkv_in = nc.declare_dram_parameter("wqkv", [L, MT_QKV, 128, E], BF16, isOutput=False)
    bqkv_in = nc.declare_dram_parameter("bqkv", [L, 3 * E], F32, isOutput=False)
    wo_in = nc.declare_dram_parameter("wo", [L, E, E], BF16, isOutput=False)
    w1_in = nc.declare_dram_parameter("w1", [L, MT_FF, 128, E], BF16, isOutput=False)
    b1_in = nc.declare_dram_parameter("b1", [L, FF], F32, isOutput=False)
    w2_in = nc.declare_dram_parameter("w2", [L, FF, E], BF16, isOutput=False)
    mask_in = nc.declare_dram_parameter("masks", [NB, 128, 256], BF16, isOutput=False)
    # pre-tiled: [NCH, 128, KT*512]
    wh_in = nc.declare_dram_parameter("whead", [NCH, 128, KT * 512], BF16, isOutput=False)
    ident_in = nc.declare_dram_parameter("ident", [128, 128], BF16, isOutput=False)
    bo_in = nc.declare_dram_parameter("bo", [L, E], F32, isOutput=False) if use_bo else None
    b2_in = nc.declare_dram_parameter("b2", [L, E], F32, isOutput=False) if use_b2 else None
    out = nc.declare_dram_parameter("logits", [B * N, VP], F32, isOutput=True)

    cc_kv_in = [nc.dram_tensor(f"cckv_i{l}", [CC_KV_SZ], FP8) for l in range(L)]
    cc_kv_out = [nc.dram_tensor(f"cckv_o{l}", [4, CC_KV_SZ], FP8) for l in range(L)]
    cc_h_in = nc.dram_tensor("cch_i", [E * R], BF16)
    cc_h_out = nc.dram_tensor("cch_o", [8, E * R], BF16, addr_space="Shared")
    warm4_in = nc.dram_tensor("warm4_i", [64], BF16)
    warm4_out = nc.dram_tensor("warm4_o", [4, 64], BF16)
    warm8_in = nc.dram_tensor("warm8_i", [64], BF16)
    warm8_out = nc.dram_tensor("warm8_o", [8, 64], BF16, addr_space="Shared")

    with _TileContext(nc) as tc, ExitStack() as ctx:
        const = ctx.enter_context(tc.tile_pool(name="const", bufs=1))

        ident = const.tile([128, 128], BF16)
        nc.sync.dma_start(out=ident, in_=ident_in[:])

        h_sb = [const.tile([128, E], F32, tag=f"h{rb}", name=f"h{rb}") for rb in range(2)]
        for rb in range(2):
            nc.sync.dma_start(out=h_sb[rb], in_=h0_in[rb])

        mask_sb = const.tile([128, NB, 256], BF16)
        nc.sync.dma_start(out=mask_sb, in_=mask_in.rearrange("k p c -> p k c"))

        hfT_all = const.tile([128, KT, B * N], BF16, tag="hfT_all")

        eps_t = const.tile([128, 1], F32)
        nc.vector.memset(eps_t, EPS)

        with ExitStack() as lctx:
            p = {
                "small": lctx.enter_context(tc.tile_pool(name="small", bufs=4)),
                "work": lctx.enter_context(tc.tile_pool(name="work", bufs=3)),
                "wpool": lctx.enter_context(tc.tile_pool(name="wpool", bufs=6)),
                "big": lctx.enter_context(tc.tile_pool(name="big", bufs=1)),
                "eSp": lctx.enter_context(tc.tile_pool(name="eSp", bufs=3)),
                "ps256": lctx.enter_context(
                    tc.tile_pool(name="ps256", bufs=3, space="PSUM")),
                "psT": lctx.enter_context(
                    tc.tile_pool(name="psT", bufs=1, space="PSUM")),
                "psO": lctx.enter_context(
                    tc.tile_pool(name="psO", bufs=2, space="PSUM")),
                "psR": lctx.enter_context(
                    tc.tile_pool(name="psR", bufs=2, space="PSUM")),
            }

            def layernorm_to_T(xT_dst):
                for rb in range(2):
                    mv = p["small"].tile([128, nc.vector.BN_AGGR_DIM], F32, tag="ln_mv")
                    stats = p["small"].tile(
                        [128, 3, nc.vector.BN_STATS_DIM], F32, tag="ln_st")
                    xin = h_sb[rb]
                    for s in range(3):
                        nc.vector.bn_stats(
                            out=stats[:, s, :], in_=xin[:, s * 256:(s + 1) * 256])
                    nc.vector.bn_aggr(out=mv, in_=stats)
                    rstd = p["small"].tile([128, 1], F32, tag="ln_rstd")
                    nc.scalar.activation(
                        out=rstd, in_=mv[:, 1:2], func=AF.Sqrt, bias=eps_t, scale=1.0)
                    nc.vector.reciprocal(out=rstd, in_=rstd)
                    y = p["work"].tile([128, E], BF16, tag="ln_y")
                    nc.vector.tensor_scalar(
                        out=y, in0=xin, scalar1=mv[:, 0:1], scalar2=rstd,
                        op0=mybir.AluOpType.subtract, op1=mybir.AluOpType.mult)
                    for kt in range(KT):
                        pst = p["psT"].tile([128, 128], BF16, tag="psT")
                        nc.tensor.transpose(pst, y[:, kt * 128:(kt + 1) * 128], ident)
                        nc.vector.tensor_copy(
                            out=xT_dst[:, kt, rb * 128:(rb + 1) * 128], in_=pst)

            for l in range(L):
                # ---- LN1 -> xT ----
                xT = p["work"].tile([128, KT, R], BF16, tag="xT")
                layernorm_to_T(xT)

                bq = p["small"].tile([128, MT_QKV], F32, tag="bqkv")
                nc.sync.dma_start(
                    out=bq, in_=bqkv_in[l].rearrange("(m q) -> q m", q=128))

                kT_c = p["work"].tile([128, KT, R], FP8, tag="kT_c")
                va_c = p["work"].tile([128, 2, H * 65], BF16, tag="va_c")
                nc.vector.memset(
                    va_c.rearrange("q a (h o) -> q a h o", o=65)[:, :, :, 64:65], 1.0)
                qT = p["work"].tile([128, KT, R], BF16, tag="qT")

                def qkv_mtile(m, dst_ap):
                    wq_m = p["wpool"].tile([128, KT, 128], BF16, tag="wqkv_m")
                    nc.sync.dma_start(
                        out=wq_m,
                        in_=wqkv_in[l, m].rearrange("q (kt c) -> q kt c", kt=KT))
                    ps = p["ps256"].tile([128, R], F32, tag="mm256")
                    for kt in range(KT):
                        nc.tensor.matmul(ps, wq_m[:, kt, :], xT[:, kt, :],
                                         start=(kt == 0), stop=(kt == KT - 1))
                    nc.vector.tensor_scalar_add(
                        out=dst_ap, in0=ps, scalar1=bq[:, m:m + 1])

                # K m-tiles first (feed the collective), then V, then ship
                for mk in range(KT):
                    qkv_mtile(KT + mk, kT_c[:, mk, :])
                for mv_ in range(KT):
                    vT_t = p["work"].tile([128, R], BF16, tag="vT_t")
                    qkv_mtile(2 * KT + mv_, vT_t[:, :])
                    for rb in range(2):
                        pst = p["psT"].tile([128, 128], BF16, tag="psT")
                        nc.tensor.transpose(
                            pst, vT_t[:, rb * 128:(rb + 1) * 128], ident)
                        for hh in range(2):
                            h_abs = 2 * mv_ + hh
                            nc.vector.tensor_copy(
                                out=va_c[:, rb, h_abs * 65:h_abs * 65 + 64],
                                in_=pst[:, hh * 64:hh * 64 + 64])
                nc.sync.dma_start(
                    out=cc_kv_in[l][0:E * R].rearrange(
                        "(kt q c) -> q kt c", q=128, c=R),
                    in_=kT_c)
                nc.sync.dma_start(
                    out=cc_kv_in[l][E * R:].bitcast(BF16).rearrange(
                        "(a q c) -> q a c", q=128, a=2),
                    in_=va_c)
                nc.gpsimd.collective_compute(
                    "AllGather", mybir.AluOpType.bypass, replica_groups=KV_GROUPS,
                    ins=[cc_kv_in[l][:]], outs=[cc_kv_out[l][:]])

                # Q m-tiles while the collective runs
                for mq in range(KT):
                    qkv_mtile(mq, qT[:, mq, :])

                # ---- pull gathered K/V ----
                kT_all = p["big"].tile([128, KT, N], FP8, tag="kT_all")
                V_all = p["big"].tile([128, NB, H * 65], BF16, tag="V_all")
                kview = cc_kv_out[l][:, 0:E * R].rearrange("g (f c) -> g f c", c=R)
                vview = cc_kv_out[l][:, E * R:].bitcast(BF16).rearrange("g (r x) -> g r x", x=H * 65)
                for g in range(4):
                    for half in range(2):
                        kb = g if half == 0 else 7 - g
                        nc.sync.dma_start(
                            out=kT_all[:, :, kb * 128:(kb + 1) * 128],
                            in_=kview[g].rearrange("(kt q) c -> q kt c", q=128)[
                                :, :, half * 128:(half + 1) * 128])
                        nc.sync.dma_start(
                            out=V_all[:, kb, :],
                            in_=vview[g, half * 128:(half + 1) * 128, :])

                # ---- attention (2-head software pipeline: S runs 2 heads
                # ahead of AV so AV never stalls on exp/mask) ----
                o_pack = p["work"].tile([128, 2, E], BF16, tag="o_pack")
                eS_of = {}

                def emit_S(hh):
                    rhs_q = qT[(hh % 2) * 64:(hh % 2) * 64 + 64, hh // 2, :]
                    eS = p["eSp"].tile([128, 4 * 256 + 4 * 128], BF16, tag="eS",
                                       name=f"eS_{l}_{hh}")
                    eS_of[hh] = eS
                    for kb in range(NB):
                        wN = 256 if kb < 4 else 128
                        off = kb * 256 if kb < 4 else 1024 + (kb - 4) * 128
                        ps = p["ps256"].tile([128, R], F32, tag="mm256")
                        nc.tensor.matmul(
                            ps[:, 0:wN],
                            kT_all[(hh % 2) * 64:(hh % 2) * 64 + 64, hh // 2,
                                   kb * 128:(kb + 1) * 128],
                            rhs_q if kb < 4 else rhs_q[:, 128:256],
                            start=True, stop=True)
                        nc.scalar.activation(
                            out=eS[:, off:off + wN], in_=ps[:, 0:wN], func=AF.Exp)
                        m_sl = (mask_sb[:, kb, 0:256] if kb < 4
                                else mask_sb[:, kb, 128:256])
                        nc.vector.tensor_mul(
                            out=eS[:, off:off + wN], in0=eS[:, off:off + wN], in1=m_sl)

                def emit_AV(hh):
                    eS = eS_of.pop(hh)
                    for qb in range(2):
                        nkb = 4 if qb == 0 else NB
                        psO = p["psO"].tile([128, 65], F32, tag="psO")
                        for kb in range(nkb):
                            if kb < 4:
                                sl = eS[:, kb * 256 + qb * 128:
                                        kb * 256 + qb * 128 + 128]
                            else:
                                sl = eS[:, 1024 + (kb - 4) * 128:
                                        1024 + (kb - 4) * 128 + 128]
                            nc.tensor.matmul(
                                psO, sl, V_all[:, kb, hh * 65:(hh + 1) * 65],
                                start=(kb == 0), stop=(kb == nkb - 1))
                        recip = p["small"].tile([128, 1], F32, tag="recip")
                        nc.vector.reciprocal(out=recip, in_=psO[:, 64:65])
                        nc.vector.tensor_scalar_mul(
                            out=o_pack[:, qb, hh * 64:(hh + 1) * 64],
                            in0=psO[:, 0:64], scalar1=recip)

                emit_S(0)
                emit_S(1)
                for hh in range(H):
                    if hh + 2 < H:
                        emit_S(hh + 2)
                    emit_AV(hh)

                oT = p["work"].tile([128, KT, R], BF16, tag="oT")
                for qb in range(2):
                    for f in range(KT):
                        pst = p["psT"].tile([128, 128], BF16, tag="psT")
                        nc.tensor.transpose(
                            pst, o_pack[:, qb, f * 128:(f + 1) * 128], ident)
                        nc.vector.tensor_copy(
                            out=oT[:, f, qb * 128:(qb + 1) * 128], in_=pst)

                # ---- out_proj + residual ----
                bo_b = None
                if bo_in is not None:
                    bo_b = p["small"].tile([128, E], F32, tag="bo_b")
                    nc.sync.dma_start(out=bo_b, in_=_bcast_row(bo_in, l))
                for rb in range(2):
                    psr = [p["psR"].tile([128, 384], F32, tag="psR",
                                         name=f"psra_{l}_{rb}{i}") for i in range(2)]
                    for kt in range(KT):
                        wo_t = p["wpool"].tile([128, E], BF16, tag="wo_t")
                        nc.sync.dma_start(
                            out=wo_t, in_=wo_in[l, kt * 128:(kt + 1) * 128, :])
                        for half in range(2):
                            nc.tensor.matmul(
                                psr[half],
                                oT[:, kt, rb * 128:(rb + 1) * 128],
                                wo_t[:, half * 384:(half + 1) * 384],
                                start=(kt == 0), stop=(kt == KT - 1))
                    for half in range(2):
                        hs = h_sb[rb][:, half * 384:(half + 1) * 384]
                        nc.vector.tensor_add(out=hs, in0=hs, in1=psr[half])
                    if bo_b is not None:
                        nc.vector.tensor_add(out=h_sb[rb], in0=h_sb[rb], in1=bo_b)

                # ---- LN2 -> x2T ----
                x2T = p["work"].tile([128, KT, R], BF16, tag="x2T")
                layernorm_to_T(x2T)

                # ---- FFN1 (gelu+bias at evict) ----
                b1s = p["small"].tile([128, MT_FF], F32, tag="b1s")
                nc.sync.dma_start(
                    out=b1s, in_=b1_in[l].rearrange("(m q) -> q m", q=128))
                gT = p["big"].tile([128, MT_FF, R], BF16, tag="gT")
                for m in range(MT_FF):
                    w1_m = p["wpool"].tile([128, KT, 128], BF16, tag="w1_m")
                    nc.sync.dma_start(
                        out=w1_m,
                        in_=w1_in[l, m].rearrange("q (kt c) -> q kt c", kt=KT))
                    ps = p["ps256"].tile([128, R], F32, tag="mm256")
                    for kt in range(KT):
                        nc.tensor.matmul(ps, w1_m[:, kt, :], x2T[:, kt, :],
                                         start=(kt == 0), stop=(kt == KT - 1))
                    nc.scalar.activation(
                        out=gT[:, m, :], in_=ps, func=AF.Gelu_apprx_tanh,
                        bias=b1s[:, m:m + 1], scale=1.0)

                # ---- FFN2 + residual ----
                b2_b = None
                if b2_in is not None:
                    b2_b = p["small"].tile([128, E], F32, tag="b2_b")
                    nc.sync.dma_start(out=b2_b, in_=_bcast_row(b2_in, l))
                for rb in range(2):
                    psr = [p["psR"].tile([128, 384], F32, tag="psR",
                                         name=f"psrb_{l}_{rb}{i}") for i in range(2)]
                    for kf in range(MT_FF):
                        w2_k = p["wpool"].tile([128, E], BF16, tag="w2_k")
                        nc.sync.dma_start(
                            out=w2_k, in_=w2_in[l, kf * 128:(kf + 1) * 128, :])
                        for half in range(2):
                            nc.tensor.matmul(
                                psr[half],
                                gT[:, kf, rb * 128:(rb + 1) * 128],
                                w2_k[:, half * 384:(half + 1) * 384],
                                start=(kf == 0), stop=(kf == MT_FF - 1))
                    for half in range(2):
                        hs = h_sb[rb][:, half * 384:(half + 1) * 384]
                        nc.vector.tensor_add(out=hs, in0=hs, in1=psr[half])
                    if b2_b is not None:
                        nc.vector.tensor_add(out=h_sb[rb], in0=h_sb[rb], in1=b2_b)

            # ---- final LN -> gather -> hfT_all ----
            hfT = p["work"].tile([128, KT, R], BF16, tag="xT")
            layernorm_to_T(hfT)
            nc.sync.dma_start(
                out=cc_h_in[:].rearrange("(kt q c) -> q kt c", q=128, c=R),
                in_=hfT)
            nc.gpsimd.collective_compute(
                "AllGather", mybir.AluOpType.bypass, replica_groups=H8_GROUPS,
                ins=[cc_h_in[:]], outs=[cc_h_out[:]])
            hgv = cc_h_out.rearrange("g (f c) -> g f c", c=R)
            for rt in range(16):
                bb, blk = rt // NB, rt % NB
                g = blk if blk < 4 else 7 - blk
                half = 0 if blk < 4 else 1
                nc.sync.dma_start(
                    out=hfT_all[:, :, rt * 128:(rt + 1) * 128],
                    in_=hgv[bb * 4 + g].rearrange("(kt q) c -> q kt c", q=128)[
                        :, :, half * 128:(half + 1) * 128])

        # ---- lm_head ----
        with ExitStack() as hctx:
            whp = hctx.enter_context(tc.tile_pool(name="whp", bufs=3))
            lsb = hctx.enter_context(tc.tile_pool(name="lsb", bufs=6))
            psH = hctx.enter_context(tc.tile_pool(name="psH", bufs=6, space="PSUM"))
            for nch in range(NCH):
                wN = 512 if nch < NCH - 1 else VP - 512 * (NCH - 1)
                wh = whp.tile([128, KT, 512], BF16, tag="wh")
                nc.sync.dma_start(
                    out=wh,
                    in_=wh_in[nch].rearrange("q (kt c) -> q kt c", kt=KT))
                for rt in range(16):
                    ps = psH.tile([128, 512], F32, tag="psH")
                    for kt in range(KT):
                        nc.tensor.matmul(
                            ps[:, 0:wN], hfT_all[:, kt, rt * 128:(rt + 1) * 128],
                            wh[:, kt, 0:wN], start=(kt == 0), stop=(kt == KT - 1))
                    ls = lsb.tile([128, 512], F32, tag="ls")
                    if rt % 2 == 0:
                        nc.vector.tensor_copy(out=ls[:, 0:wN], in_=ps[:, 0:wN])
                    else:
                        nc.scalar.activation(
                            out=ls[:, 0:wN], in_=ps[:, 0:wN], func=AF.Copy)
                    nc.sync.dma_start(
                        out=out[rt * 128:(rt + 1) * 128, nch * 512:nch * 512 + wN],
                        in_=ls[:, 0:wN])
    _split_multi_waits(nc)
    return nc


# ---------------------------------------------------------------------------
# host side
# ---------------------------------------------------------------------------

def _sinusoidal_pos(n, dim):
    pos = np.arange(n, dtype=np.float32)[:, None]
    i = np.arange(0, dim, 2, dtype=np.float32)
    j = np.arange(1, dim, 2, dtype=np.float32)
    s = np.sin(pos / np.power(np.float32(10000.0), 2.0 * i / dim, dtype=np.float32))
    c = np.cos(pos / np.power(np.float32(10000.0), 2.0 * j / dim, dtype=np.float32))
    return np.stack([s, c], axis=-1).reshape(n, dim).astype(np.float32)


_CACHE = {}


def _get_nc(use_bo, use_b2):
    key = (use_bo, use_b2)
    if key not in _CACHE:
        _CACHE[key] = build_nc(use_bo, use_b2)
    return _CACHE[key]


def _tile_w(w):
    """[E, M*128] -> [M, 128, KT*128]: [m, p, kt*128+c] = w[kt*128+p, m*128+c]."""
    M = w.shape[1] // 128
    return np.ascontiguousarray(
        w.reshape(KT, 128, M, 128).transpose(2, 1, 0, 3).reshape(M, 128, KT * 128))


def kernel(x, tok_emb, wq, wk, wv, wo, bo, ln1_g, ln1_b, ln2_g, ln2_b,
           w1, b1, w2, b2, lnf_g, lnf_b, w_head, _trace=False):
    x = np.asarray(x)
    f = lambda a: np.asarray(a, dtype=np.float32)
    tok_emb, wq, wk, wv, wo = f(tok_emb), f(wq), f(wk), f(wv), f(wo)
    bo, w1, b1, w2, b2 = f(bo), f(w1), f(b1), f(w2), f(b2)
    ln1_g, ln1_b, ln2_g, ln2_b = f(ln1_g), f(ln1_b), f(ln2_g), f(ln2_b)
    lnf_g, lnf_b, w_head = f(lnf_g), f(lnf_b), f(w_head)

    h0 = tok_emb[x] + _sinusoidal_pos(N, E)[None, :, :]     # [B, N, E] f32

    scale = np.float32(1.0 / np.sqrt(HD))
    wqkv = np.concatenate([wq * scale, wk, wv], axis=2)      # [L, E, 3E]
    bqkv = np.einsum("le,lef->lf", ln1_b, wqkv).astype(np.float32)
    wqkv = (ln1_g[:, :, None] * wqkv).astype(BF)
    wqkv_t = np.stack([_tile_w(wqkv[l]) for l in range(L)])
    b1c = (b1 + np.einsum("le,lef->lf", ln2_b, w1)).astype(np.float32)
    w1f = (ln2_g[:, :, None] * w1).astype(BF)
    w1_t = np.stack([_tile_w(w1f[l]) for l in range(L)])
    w2f = np.ascontiguousarray(w2.astype(BF))
    wof = np.ascontiguousarray(wo.astype(BF))
    whf = np.zeros((E, 8 * VPAD), dtype=np.float32)
    wh_scaled = lnf_g[:, None] * w_head
    for c in range(8):
        lo, hi = c * VP, min((c + 1) * VP, V)
        whf[:, c * VPAD:c * VPAD + (hi - lo)] = wh_scaled[:, lo:hi]
    whf = whf.astype(BF)

    use_bo = bool(np.any(bo))
    use_b2 = bool(np.any(b2))
    nc = _get_nc(use_bo, use_b2)

    ident = np.eye(128, dtype=BF)
    key_idx = np.arange(N)[:, None]
    in_maps = []
    for c in range(8):
        bb, g = c // 4, c % 4
        blocks = [g, 7 - g]
        h0c = np.stack([h0[bb, blk * 128:(blk + 1) * 128, :] for blk in blocks])
        masks = np.zeros((NB, 128, 256), dtype=BF)
        for qi, blk in enumerate(blocks):
            q = blk * 128 + np.arange(128)[None, :]
            allow = (key_idx <= q).astype(np.float32).reshape(NB, 128, 128)
            masks[:, :, qi * 128:(qi + 1) * 128] = allow.astype(BF)
        # whead slice, re-tiled to [NCH, 128, KT*512]
        whc = whf[:, c * VPAD:(c + 1) * VPAD]
        whc_t = np.ascontiguousarray(
            whc.reshape(KT, 128, NCH, 512).transpose(2, 1, 0, 3).reshape(
                NCH, 128, KT * 512))
        m = {
            "h0": np.ascontiguousarray(h0c, dtype=np.float32),
            "wqkv": wqkv_t, "bqkv": bqkv, "wo": wof,
            "w1": w1_t, "b1": b1c, "w2": w2f,
            "masks": masks, "whead": whc_t, "ident": ident,
        }
        if use_bo:
            m["bo"] = bo
        if use_b2:
            m["b2"] = b2
        in_maps.append(m)

    res = run_bass_kernel_spmd(nc, in_maps, list(range(8)), trace=_trace)
    logits = np.concatenate([res.results[c]["logits"] for c in range(8)], axis=1)
    logits = logits[:, :V]
    if np.any(lnf_b):
        logits = logits + (lnf_b @ w_head)[None, :]
    out = logits.reshape(B, N, V)
    if _trace:
        return out, res
    return out



# revision 4
# speedup vs baseline: 1.0795x; 1.0795x over previous
"""GPT forward (6-layer, E=768, H=12, N=1024, B=2, V=50257) on 8 TRN2 cores.

Sharding: sequence-sharded layers (cores 0-3 batch 0, cores 4-7 batch 1;
core in-group index g owns row-blocks g and 7-g of its batch), one fused K/V
AllGather per layer within each 4-core group, a split (per row-block) 8-core
AllGather of the final LN output, then a vocab-sharded lm_head (each core
computes all 2048 rows x a 6283-wide vocab slice; host concatenates slices).

v2 scheduling changes vs v1:
- whole-tensor weight DMAs (wqkv 3 chunks, wo/w1 1 each, w2 4 chunks, whead 1)
  instead of per-m-tile loads: ~10 DMA issues/layer instead of ~120, and
  w2/wo are no longer fetched twice per layer.
- kf-outer FFN2 / kt-outer out_proj with 4 live PSUM accumulators.
- one shared 8-slot PSUM ring for all layer-phase accumulation/transposes.
- S psums merged into 512-wide banks: 3 exps + 3 mask-muls per head
  instead of 8 each.
- lm_head: whead preloaded in one DMA before the final AllGather; the final
  AllGather is split per row-block so the head starts on the first half's
  rows while the second gathers; logits written bf16 (host upcasts).

Compute: bf16 matmul inputs, fp32 PSUM/residual/LN. LN gammas are folded into
the following weights host-side, betas into biases. The causal mask is
multiplicative bf16 applied after exp (scores are small, so softmax's
max-subtraction is safely skipped); softmax denominators come from an
appended ones-column on V.
"""

from contextlib import ExitStack

import numpy as np
import ml_dtypes

import concourse.bass as bass
import concourse.mybir as mybir
import concourse.tile as tile
from concourse.bass_utils import run_bass_kernel_spmd
from concourse.vector_clock import ScopedClock, VectorClock

F32 = mybir.dt.float32
BF16 = mybir.dt.bfloat16
AF = mybir.ActivationFunctionType
BF = ml_dtypes.bfloat16
FP8 = mybir.dt.float8e4

V, E, N, H, L = 50257, 768, 1024, 12, 6
HD = E // H          # 64
FF = 4 * E           # 3072
B = 2
KT = E // 128         # 6 feature k-tiles
MT_QKV = 3 * KT       # 18 m-tiles for fused qkv
MT_FF = FF // 128     # 24
R = 256               # rows per core
NB = 8                # 128-row blocks per batch
VP = 6283             # vocab slice per core (8*6283 = 50264 >= V)
VPAD = 13 * 512       # host-side padded vocab slice (6656)
NCH = 13              # head vocab chunks of 512 (last used width = 139)
KV_GROUPS = [[0, 1, 2, 3], [4, 5, 6, 7]]
H8_GROUPS = [[0, 1, 2, 3, 4, 5, 6, 7]]
CC_KV_SZ = E * R + 2 * R * (H * 65)  # bytes: K as fp8, V as bf16
EPS = 1e-5


class _TileContext(tile.TileContext):
    """This image's walrus rejects Drain instructions with >1 sync-wait.
    Split the kernel-tail drain into one Drain per pending proc."""

    def _drain_and_barrier(self, tick_clock, wait_clock):
        nc = self.nc
        vec = tick_clock.global_clock
        n = len(vec)
        for proc in range(n):
            t = vec[proc]
            if t <= 0:
                continue
            sub = VectorClock([t if i == proc else 0 for i in range(n)])
            d = nc.sync.drain()
            wait_clock.add_sem_waits(d.ins, ScopedClock({None: sub}))
        nc.sync.drain()
        nc.all_engine_barrier()
        assert self.sems is not None
        popped = nc._tile_sem_poison_stack.pop()
        assert popped is self._sem_poison
        nc.clear_and_free_semaphores(list(self.sems.allocated().values()))
        nc.all_engine_barrier()


def _split_multi_waits(nc):
    """This walrus build encodes at most one sync-wait per instruction.
    Hoist extra waits onto NoOps inserted just before, on the same engine."""
    ctr = 0
    for bb in nc.main_func.blocks:
        il = bb.instructions
        out_l = []
        for ins in il:
            si = ins.sync_info
            if si is not None and si.on_wait is not None and len(si.on_wait) > 1:
                waits = list(si.on_wait)
                for w in waits[:-1]:
                    noop = mybir.InstNoOp(name=f"wsplit_{ctr}", ins=[], outs=[])
                    ctr += 1
                    noop.engine = ins.engine
                    noop.sync_info = type(si)(on_wait=[w], on_update=[])
                    out_l.append(noop)
                si.on_wait = waits[-1:]
            out_l.append(ins)
        il[:] = out_l


def _bcast_row(t, row, p=128):
    """AP reading DRAM row `t[row]` broadcast across p partitions."""
    base = t[row] if row is not None else t[:]
    return bass.AP(
        tensor=base.tensor, offset=base.offset,
        ap=[[0, p]] + [list(x) for x in base.ap])


def build_nc(use_bo, use_b2):
    nc = bass.Bass(num_devices=8)

    h0_in = nc.declare_dram_parameter("h0", [2, 128, E], F32, isOutput=False)
    # pre-tiled: [L, 18, 128, KT*128]  (m-tile, partition=feat%128, kt*128+mcol)
    wqkv_in = nc.declare_dram_parameter("wqkv", [L, MT_QKV, 128, E], BF16, isOutput=False)
    bqkv_in = nc.declare_dram_parameter("bqkv", [L, 3 * E], F32, isOutput=False)
    wo_in = nc.declare_dram_parameter("wo", [L, E, E], BF16, isOutput=False)
    w1_in = nc.declare_dram_parameter("w1", [L, MT_FF, 128, E], BF16, isOutput=False)
    b1_in = nc.declare_dram_parameter("b1", [L, FF], F32, isOutput=False)
    w2_in = nc.declare_dram_parameter("w2", [L, FF, E], BF16, isOutput=False)
    mask_in = nc.declare_dram_parameter("masks", [NB, 128, 256], BF16, isOutput=False)
    # pre-tiled: [NCH, 128, KT*512]
    wh_in = nc.declare_dram_parameter("whead", [NCH, 128, KT * 512], BF16, isOutput=False)
    ident_in = nc.declare_dram_parameter("ident", [128, 128], BF16, isOutput=False)
    bo_in = nc.declare_dram_parameter("bo", [L, E], F32, isOutput=False) if use_bo else None
    b2_in = nc.declare_dram_parameter("b2", [L, E], F32, isOutput=False) if use_b2 else None
    out = nc.declare_dram_parameter("logits", [B * N, VP], BF16, isOutput=True)

    cc_kv_in = [nc.dram_tensor(f"cckv_i{l}", [CC_KV_SZ], FP8) for l in range(L)]
    cc_kv_out = [nc.dram_tensor(f"cckv_o{l}", [4, CC_KV_SZ], FP8) for l in range(L)]
    cch_i = [nc.dram_tensor(f"cch_i{rb}", [E * 128], BF16) for rb in range(2)]
    cch_o = [nc.dram_tensor(f"cch_o{rb}", [8, E * 128], BF16, addr_space="Shared")
             for rb in range(2)]

    with _TileContext(nc) as tc, ExitStack() as ctx:
        const = ctx.enter_context(tc.tile_pool(name="const", bufs=1))

        ident = const.tile([128, 128], BF16)
        nc.sync.dma_start(out=ident, in_=ident_in[:])

        h_sb = [const.tile([128, E], F32, tag=f"h{rb}", name=f"h{rb}") for rb in range(2)]
        for rb in range(2):
            nc.sync.dma_start(out=h_sb[rb], in_=h0_in[rb])

        mask_sb = const.tile([128, NB, 256], BF16)
        nc.sync.dma_start(out=mask_sb, in_=mask_in.rearrange("k p c -> p k c"))

        hfT_all = const.tile([128, KT, B * N], BF16, tag="hfT_all")

        eps_t = const.tile([128, 1], F32)
        nc.vector.memset(eps_t, EPS)

        with ExitStack() as lctx:
            p = {
                "small": lctx.enter_context(tc.tile_pool(name="small", bufs=2)),
                "wgt": lctx.enter_context(tc.tile_pool(name="wgt", bufs=1)),
                "wch": lctx.enter_context(tc.tile_pool(name="wch", bufs=2)),
                "act": lctx.enter_context(tc.tile_pool(name="act", bufs=1)),
                "act2": lctx.enter_context(tc.tile_pool(name="act2", bufs=2)),
                "big": lctx.enter_context(tc.tile_pool(name="big", bufs=1)),
                "eSp": lctx.enter_context(tc.tile_pool(name="eSp", bufs=3)),
                "ps": lctx.enter_context(
                    tc.tile_pool(name="ps", bufs=8, space="PSUM")),
            }

            def psb(name, shape=(128, 512), dtype=F32):
                return p["ps"].tile(list(shape), dtype, tag="bank", name=name)

            def layernorm_to_T(xT_dst):
                for rb in range(2):
                    mv = p["small"].tile([128, nc.vector.BN_AGGR_DIM], F32, tag="ln_mv")
                    stats = p["small"].tile(
                        [128, 3, nc.vector.BN_STATS_DIM], F32, tag="ln_st")
                    xin = h_sb[rb]
                    for s in range(3):
                        nc.vector.bn_stats(
                            out=stats[:, s, :], in_=xin[:, s * 256:(s + 1) * 256])
                    nc.vector.bn_aggr(out=mv, in_=stats)
                    rstd = p["small"].tile([128, 1], F32, tag="ln_rstd")
                    nc.scalar.activation(
                        out=rstd, in_=mv[:, 1:2], func=AF.Sqrt, bias=eps_t, scale=1.0)
                    nc.vector.reciprocal(out=rstd, in_=rstd)
                    y = p["act2"].tile([128, E], BF16, tag="ln_y")
                    nc.vector.tensor_scalar(
                        out=y, in0=xin, scalar1=mv[:, 0:1], scalar2=rstd,
                        op0=mybir.AluOpType.subtract, op1=mybir.AluOpType.mult)
                    for kt in range(KT):
                        pst = psb("psT", (128, 128), BF16)
                        nc.tensor.transpose(pst, y[:, kt * 128:(kt + 1) * 128], ident)
                        nc.vector.tensor_copy(
                            out=xT_dst[:, kt, rb * 128:(rb + 1) * 128], in_=pst)

            for l in range(L):
                # ---- weight prefetch: K chunk, V chunk, Q chunk ----
                wqkv_sb = p["wgt"].tile([128, MT_QKV, E], BF16, tag="wqkv")
                for c0, c1 in ((KT, 2 * KT), (2 * KT, 3 * KT), (0, KT)):
                    nc.sync.dma_start(
                        out=wqkv_sb[:, c0:c1, :],
                        in_=wqkv_in[l, c0:c1].rearrange("m q c -> q m c"))
                bq = p["small"].tile([128, MT_QKV], F32, tag="bqkv")
                nc.sync.dma_start(
                    out=bq, in_=bqkv_in[l].rearrange("(m q) -> q m", q=128))

                # ---- LN1 -> xT ----
                xT = p["act"].tile([128, KT, R], BF16, tag="xT")
                layernorm_to_T(xT)

                kT_c = p["act"].tile([128, KT, R], FP8, tag="kT_c")
                va_c = p["act"].tile([128, 2, H * 65], BF16, tag="va_c")
                nc.vector.memset(
                    va_c.rearrange("q a (h o) -> q a h o", o=65)[:, :, :, 64:65], 1.0)
                qT = p["act"].tile([128, KT, R], BF16, tag="qT")

                def qkv_mtile(m, dst_ap):
                    ps = psb("mmQKV")
                    for kt in range(KT):
                        nc.tensor.matmul(
                            ps[:, 0:R], wqkv_sb[:, m, kt * 128:(kt + 1) * 128],
                            xT[:, kt, :], start=(kt == 0), stop=(kt == KT - 1))
                    nc.vector.tensor_scalar_add(
                        out=dst_ap, in0=ps[:, 0:R], scalar1=bq[:, m:m + 1])

                # K m-tiles first (feed the collective), then V, then ship
                for mk in range(KT):
                    qkv_mtile(KT + mk, kT_c[:, mk, :])
                for mv_ in range(KT):
                    vT_t = p["act2"].tile([128, R], BF16, tag="vT_t")
                    qkv_mtile(2 * KT + mv_, vT_t[:, :])
                    for rb in range(2):
                        pst = psb("psT", (128, 128), BF16)
                        nc.tensor.transpose(
                            pst, vT_t[:, rb * 128:(rb + 1) * 128], ident)
                        for hh in range(2):
                            h_abs = 2 * mv_ + hh
                            nc.vector.tensor_copy(
                                out=va_c[:, rb, h_abs * 65:h_abs * 65 + 64],
                                in_=pst[:, hh * 64:hh * 64 + 64])
                nc.sync.dma_start(
                    out=cc_kv_in[l][0:E * R].rearrange(
                        "(kt q c) -> q kt c", q=128, c=R),
                    in_=kT_c)
                nc.sync.dma_start(
                    out=cc_kv_in[l][E * R:].bitcast(BF16).rearrange(
                        "(a q c) -> q a c", q=128, a=2),
                    in_=va_c)
                nc.gpsimd.collective_compute(
                    "AllGather", mybir.AluOpType.bypass, replica_groups=KV_GROUPS,
                    ins=[cc_kv_in[l][:]], outs=[cc_kv_out[l][:]])

                # weight DMAs for the rest of the layer (issued before the
                # collective-gated pulls so they stream during attention)
                wo_sb = p["wgt"].tile([128, KT, E], BF16, tag="wo")
                nc.sync.dma_start(
                    out=wo_sb, in_=wo_in[l].rearrange("(kt q) c -> q kt c", q=128))
                w1_sb = p["wgt"].tile([128, MT_FF, E], BF16, tag="w1")
                nc.sync.dma_start(
                    out=w1_sb, in_=w1_in[l].rearrange("m q c -> q m c"))
                b1s = p["small"].tile([128, MT_FF], F32, tag="b1s")
                nc.sync.dma_start(
                    out=b1s, in_=b1_in[l].rearrange("(m q) -> q m", q=128))
                bo_b = None
                if bo_in is not None:
                    bo_b = p["small"].tile([128, E], F32, tag="bo_b")
                    nc.sync.dma_start(out=bo_b, in_=_bcast_row(bo_in, l))

                # Q m-tiles while the collective runs
                for mq in range(KT):
                    qkv_mtile(mq, qT[:, mq, :])

                # ---- pull gathered K/V ----
                kT_all = p["big"].tile([128, KT, N], FP8, tag="kT_all")
                V_all = p["big"].tile([128, NB, H * 65], BF16, tag="V_all")
                kview = cc_kv_out[l][:, 0:E * R].rearrange("g (f c) -> g f c", c=R)
                vview = cc_kv_out[l][:, E * R:].bitcast(BF16).rearrange(
                    "g (r x) -> g r x", x=H * 65)
                for g in range(4):
                    for half in range(2):
                        kb = g if half == 0 else 7 - g
                        nc.sync.dma_start(
                            out=kT_all[:, :, kb * 128:(kb + 1) * 128],
                            in_=kview[g].rearrange("(kt q) c -> q kt c", q=128)[
                                :, :, half * 128:(half + 1) * 128])
                        nc.sync.dma_start(
                            out=V_all[:, kb, :],
                            in_=vview[g, half * 128:(half + 1) * 128, :])

                # ---- attention (2-head software pipeline: S runs 2 heads
                # ahead of AV so AV never stalls on exp/mask) ----
                o_pack = p["act"].tile([128, 2, E], BF16, tag="o_pack")
                eS_of = {}

                def emit_S(hh):
                    base = (hh % 2) * 64
                    rhs_q = qT[base:base + 64, hh // 2, :]
                    eS = p["eSp"].tile([128, 4 * 256 + 4 * 128], BF16, tag="eS",
                                       name=f"eS_{l}_{hh}")
                    eS_of[hh] = eS
                    # kb pairs (0,1) and (2,3): 256-wide q, one 512 psum each
                    for pr in range(2):
                        ps = psb("mmS")
                        for j in range(2):
                            kb = 2 * pr + j
                            nc.tensor.matmul(
                                ps[:, j * 256:(j + 1) * 256],
                                kT_all[base:base + 64, hh // 2,
                                       kb * 128:(kb + 1) * 128],
                                rhs_q, start=True, stop=True)
                        nc.scalar.activation(
                            out=eS[:, pr * 512:(pr + 1) * 512], in_=ps, func=AF.Exp)
                        nc.vector.tensor_mul(
                            out=eS[:, pr * 512:(pr + 1) * 512],
                            in0=eS[:, pr * 512:(pr + 1) * 512],
                            in1=mask_sb[:, 2 * pr:2 * pr + 2, :])
                    # kb 4..7: 128-wide (second q block only), one 512 psum
                    ps = psb("mmS")
                    for j in range(4):
                        nc.tensor.matmul(
                            ps[:, j * 128:(j + 1) * 128],
                            kT_all[base:base + 64, hh // 2,
                                   (4 + j) * 128:(5 + j) * 128],
                            rhs_q[:, 128:256], start=True, stop=True)
                    nc.scalar.activation(
                        out=eS[:, 1024:1536], in_=ps, func=AF.Exp)
                    nc.vector.tensor_mul(
                        out=eS[:, 1024:1536], in0=eS[:, 1024:1536],
                        in1=mask_sb[:, 4:8, 128:256])

                def emit_AV(hh):
                    eS = eS_of.pop(hh)
                    for qb in range(2):
                        nkb = 4 if qb == 0 else NB
                        psO = psb("psO", (128, 65))
                        for kb in range(nkb):
                            if kb < 4:
                                sl = eS[:, kb * 256 + qb * 128:
                                        kb * 256 + qb * 128 + 128]
                            else:
                                sl = eS[:, 1024 + (kb - 4) * 128:
                                        1024 + (kb - 4) * 128 + 128]
                            nc.tensor.matmul(
                                psO, sl, V_all[:, kb, hh * 65:(hh + 1) * 65],
                                start=(kb == 0), stop=(kb == nkb - 1))
                        recip = p["small"].tile([128, 1], F32, tag="recip")
                        nc.vector.reciprocal(out=recip, in_=psO[:, 64:65])
                        nc.vector.tensor_scalar_mul(
                            out=o_pack[:, qb, hh * 64:(hh + 1) * 64],
                            in0=psO[:, 0:64], scalar1=recip)

                emit_S(0)
                emit_S(1)
                for hh in range(H):
                    if hh + 2 < H:
                        emit_S(hh + 2)
                    emit_AV(hh)

                oT = p["act"].tile([128, KT, R], BF16, tag="oT")
                for qb in range(2):
                    for f in range(KT):
                        pst = psb("psT", (128, 128), BF16)
                        nc.tensor.transpose(
                            pst, o_pack[:, qb, f * 128:(f + 1) * 128], ident)
                        nc.vector.tensor_copy(
                            out=oT[:, f, qb * 128:(qb + 1) * 128], in_=pst)

                # ---- out_proj + residual (kt-outer: wo read once) ----
                psr = [psb(f"psra_{l}_{i}", (128, 384)) for i in range(4)]
                for kt in range(KT):
                    for rb in range(2):
                        for half in range(2):
                            nc.tensor.matmul(
                                psr[2 * rb + half],
                                oT[:, kt, rb * 128:(rb + 1) * 128],
                                wo_sb[:, kt, half * 384:(half + 1) * 384],
                                start=(kt == 0), stop=(kt == KT - 1))
                for rb in range(2):
                    for half in range(2):
                        hs = h_sb[rb][:, half * 384:(half + 1) * 384]
                        nc.vector.tensor_add(out=hs, in0=hs, in1=psr[2 * rb + half])
                    if bo_b is not None:
                        nc.vector.tensor_add(out=h_sb[rb], in0=h_sb[rb], in1=bo_b)

                # ---- LN2 -> x2T ----
                x2T = p["act"].tile([128, KT, R], BF16, tag="x2T")
                layernorm_to_T(x2T)

                b2_b = None
                if b2_in is not None:
                    b2_b = p["small"].tile([128, E], F32, tag="b2_b")
                    nc.sync.dma_start(out=b2_b, in_=_bcast_row(b2_in, l))

                # ---- FFN1 (gelu+bias at evict) ----
                gT = p["big"].tile([128, MT_FF, R], BF16, tag="gT")
                for m in range(MT_FF):
                    ps = psb("mmF1")
                    for kt in range(KT):
                        nc.tensor.matmul(
                            ps[:, 0:R], w1_sb[:, m, kt * 128:(kt + 1) * 128],
                            x2T[:, kt, :], start=(kt == 0), stop=(kt == KT - 1))
                    nc.scalar.activation(
                        out=gT[:, m, :], in_=ps[:, 0:R], func=AF.Gelu_apprx_tanh,
                        bias=b1s[:, m:m + 1], scale=1.0)

                # ---- FFN2 + residual (kf-outer: w2 read once, 4 chunks) ----
                psf = [psb(f"psrb_{l}_{i}", (128, 384)) for i in range(4)]
                for ch in range(4):
                    w2c = p["wch"].tile([128, KT, E], BF16, tag="w2ch")
                    nc.sync.dma_start(
                        out=w2c,
                        in_=w2_in[l, ch * 768:(ch + 1) * 768, :].rearrange(
                            "(kf q) c -> q kf c", q=128))
                    for k6 in range(KT):
                        kf = ch * KT + k6
                        for rb in range(2):
                            for half in range(2):
                                nc.tensor.matmul(
                                    psf[2 * rb + half],
                                    gT[:, kf, rb * 128:(rb + 1) * 128],
                                    w2c[:, k6, half * 384:(half + 1) * 384],
                                    start=(kf == 0), stop=(kf == MT_FF - 1))
                for rb in range(2):
                    for half in range(2):
                        hs = h_sb[rb][:, half * 384:(half + 1) * 384]
                        nc.vector.tensor_add(out=hs, in0=hs, in1=psf[2 * rb + half])
                    if b2_b is not None:
                        nc.vector.tensor_add(out=h_sb[rb], in0=h_sb[rb], in1=b2_b)

        # ---- lm_head: preload whead, split final AllGather per row block ----
        with ExitStack() as hctx:
            whp = hctx.enter_context(tc.tile_pool(name="whp", bufs=1))
            hsm = hctx.enter_context(tc.tile_pool(name="hsm", bufs=2))
            lsb = hctx.enter_context(tc.tile_pool(name="lsb", bufs=6))
            psH = hctx.enter_context(tc.tile_pool(name="psH", bufs=8, space="PSUM"))

            def psh(name, shape=(128, 512), dtype=F32):
                return psH.tile(list(shape), dtype, tag="bank", name=name)

            wh_all = whp.tile([128, NCH, KT * 512], BF16, tag="wh")
            nc.sync.dma_start(out=wh_all, in_=wh_in.rearrange("n q c -> q n c"))

            # final LN per row block -> ship -> AllGather (split per rb)
            hfT = hsm.tile([128, KT, R], BF16, tag="hfT", bufs=1)
            for rb in range(2):
                mv = hsm.tile([128, nc.vector.BN_AGGR_DIM], F32, tag="ln_mv")
                stats = hsm.tile([128, 3, nc.vector.BN_STATS_DIM], F32, tag="ln_st")
                for s in range(3):
                    nc.vector.bn_stats(
                        out=stats[:, s, :], in_=h_sb[rb][:, s * 256:(s + 1) * 256])
                nc.vector.bn_aggr(out=mv, in_=stats)
                rstd = hsm.tile([128, 1], F32, tag="ln_rstd")
                nc.scalar.activation(
                    out=rstd, in_=mv[:, 1:2], func=AF.Sqrt, bias=eps_t, scale=1.0)
                nc.vector.reciprocal(out=rstd, in_=rstd)
                y = hsm.tile([128, E], BF16, tag="ln_y")
                nc.vector.tensor_scalar(
                    out=y, in0=h_sb[rb], scalar1=mv[:, 0:1], scalar2=rstd,
                    op0=mybir.AluOpType.subtract, op1=mybir.AluOpType.mult)
                for kt in range(KT):
                    pst = psh("psTh", (128, 128), BF16)
                    nc.tensor.transpose(pst, y[:, kt * 128:(kt + 1) * 128], ident)
                    nc.vector.tensor_copy(
                        out=hfT[:, kt, rb * 128:(rb + 1) * 128], in_=pst)
                nc.sync.dma_start(
                    out=cch_i[rb][:].rearrange("(kt q c) -> q kt c", q=128, c=128),
                    in_=hfT[:, :, rb * 128:(rb + 1) * 128])
                nc.gpsimd.collective_compute(
                    "AllGather", mybir.AluOpType.bypass, replica_groups=H8_GROUPS,
                    ins=[cch_i[rb][:]], outs=[cch_o[rb][:]])

            # pulls: rb0 gives global blocks (bb*8+g), rb1 gives (bb*8+7-g)
            rt_of = {0: [], 1: []}
            for rb in range(2):
                hgv = cch_o[rb].rearrange("g (f c) -> g f c", c=128)
                for src in range(8):
                    bb, g = src // 4, src % 4
                    rt = bb * NB + (g if rb == 0 else 7 - g)
                    rt_of[rb].append(rt)
                    nc.sync.dma_start(
                        out=hfT_all[:, :, rt * 128:(rt + 1) * 128],
                        in_=hgv[src].rearrange("(kt q) c -> q kt c", q=128))

            # head: first the rb0 rows (available first), then rb1 rows
            for i, rt in enumerate(rt_of[0] + rt_of[1]):
                for nch in range(NCH):
                    wN = 512 if nch < NCH - 1 else VP - 512 * (NCH - 1)
                    ps = psh("psHmm")
                    for kt in range(KT):
                        nc.tensor.matmul(
                            ps[:, 0:wN], hfT_all[:, kt, rt * 128:(rt + 1) * 128],
                            wh_all[:, nch, kt * 512:kt * 512 + wN],
                            start=(kt == 0), stop=(kt == KT - 1))
                    ls = lsb.tile([128, 512], BF16, tag="ls")
                    if nch % 2 == 0:
                        nc.vector.tensor_copy(out=ls[:, 0:wN], in_=ps[:, 0:wN])
                    else:
                        nc.scalar.activation(
                            out=ls[:, 0:wN], in_=ps[:, 0:wN], func=AF.Copy)
                    nc.sync.dma_start(
                        out=out[rt * 128:(rt + 1) * 128, nch * 512:nch * 512 + wN],
                        in_=ls[:, 0:wN])
    _split_multi_waits(nc)
    return nc


# ---------------------------------------------------------------------------
# host side
# ---------------------------------------------------------------------------

def _sinusoidal_pos(n, dim):
    pos = np.arange(n, dtype=np.float32)[:, None]
    i = np.arange(0, dim, 2, dtype=np.float32)
    j = np.arange(1, dim, 2, dtype=np.float32)
    s = np.sin(pos / np.power(np.float32(10000.0), 2.0 * i / dim, dtype=np.float32))
    c = np.cos(pos / np.power(np.float32(10000.0), 2.0 * j / dim, dtype=np.float32))
    return np.stack([s, c], axis=-1).reshape(n, dim).astype(np.float32)


_CACHE = {}


def _get_nc(use_bo, use_b2):
    key = (use_bo, use_b2)
    if key not in _CACHE:
        _CACHE[key] = build_nc(use_bo, use_b2)
    return _CACHE[key]


def _tile_w(w):
    """[E, M*128] -> [M, 128, KT*128]: [m, p, kt*128+c] = w[kt*128+p, m*128+c]."""
    M = w.shape[1] // 128
    return np.ascontiguousarray(
        w.reshape(KT, 128, M, 128).transpose(2, 1, 0, 3).reshape(M, 128, KT * 128))


def kernel(x, tok_emb, wq, wk, wv, wo, bo, ln1_g, ln1_b, ln2_g, ln2_b,
           w1, b1, w2, b2, lnf_g, lnf_b, w_head, _trace=False):
    x = np.asarray(x)
    f = lambda a: np.asarray(a, dtype=np.float32)
    tok_emb, wq, wk, wv, wo = f(tok_emb), f(wq), f(wk), f(wv), f(wo)
    bo, w1, b1, w2, b2 = f(bo), f(w1), f(b1), f(w2), f(b2)
    ln1_g, ln1_b, ln2_g, ln2_b = f(ln1_g), f(ln1_b), f(ln2_g), f(ln2_b)
    lnf_g, lnf_b, w_head = f(lnf_g), f(lnf_b), f(w_head)

    h0 = tok_emb[x] + _sinusoidal_pos(N, E)[None, :, :]     # [B, N, E] f32

    scale = np.float32(1.0 / np.sqrt(HD))
    wqkv = np.concatenate([wq * scale, wk, wv], axis=2)      # [L, E, 3E]
    bqkv = np.einsum("le,lef->lf", ln1_b, wqkv).astype(np.float32)
    wqkv = (ln1_g[:, :, None] * wqkv).astype(BF)
    wqkv_t = np.stack([_tile_w(wqkv[l]) for l in range(L)])
    b1c = (b1 + np.einsum("le,lef->lf", ln2_b, w1)).astype(np.float32)
    w1f = (ln2_g[:, :, None] * w1).astype(BF)
    w1_t = np.stack([_tile_w(w1f[l]) for l in range(L)])
    w2f = np.ascontiguousarray(w2.astype(BF))
    wof = np.ascontiguousarray(wo.astype(BF))
    whf = np.zeros((E, 8 * VPAD), dtype=np.float32)
    wh_scaled = lnf_g[:, None] * w_head
    for c in range(8):
        lo, hi = c * VP, min((c + 1) * VP, V)
        whf[:, c * VPAD:c * VPAD + (hi - lo)] = wh_scaled[:, lo:hi]
    whf = whf.astype(BF)

    use_bo = bool(np.any(bo))
    use_b2 = bool(np.any(b2))
    nc = _get_nc(use_bo, use_b2)

    ident = np.eye(128, dtype=BF)
    key_idx = np.arange(N)[:, None]
    in_maps = []
    for c in range(8):
        bb, g = c // 4, c % 4
        blocks = [g, 7 - g]
        h0c = np.stack([h0[bb, blk * 128:(blk + 1) * 128, :] for blk in blocks])
        masks = np.zeros((NB, 128, 256), dtype=BF)
        for qi, blk in enumerate(blocks):
            q = blk * 128 + np.arange(128)[None, :]
            allow = (key_idx <= q).astype(np.float32).reshape(NB, 128, 128)
            masks[:, :, qi * 128:(qi + 1) * 128] = allow.astype(BF)
        # whead slice, re-tiled to [NCH, 128, KT*512]
        whc = whf[:, c * VPAD:(c + 1) * VPAD]
        whc_t = np.ascontiguousarray(
            whc.reshape(KT, 128, NCH, 512).transpose(2, 1, 0, 3).reshape(
                NCH, 128, KT * 512))
        m = {
            "h0": np.ascontiguousarray(h0c, dtype=np.float32),
            "wqkv": wqkv_t, "bqkv": bqkv, "wo": wof,
            "w1": w1_t, "b1": b1c, "w2": w2f,
            "masks": masks, "whead": whc_t, "ident": ident,
        }
        if use_bo:
            m["bo"] = bo
        if use_b2:
            m["b2"] = b2
        in_maps.append(m)

    res = run_bass_kernel_spmd(nc, in_maps, list(range(8)), trace=_trace)
    logits = np.concatenate(
        [res.results[c]["logits"].astype(np.float32) for c in range(8)], axis=1)
    logits = logits[:, :V]
    if np.any(lnf_b):
        logits = logits + (lnf_b @ w_head)[None, :]
    out = logits.reshape(B, N, V)
    if _trace:
        return out, res
    return out


# revision 5
# speedup vs baseline: 1.1518x; 1.0669x over previous
"""GPT forward (6-layer, E=768, H=12, N=1024, B=2, V=50257) on 8 TRN2 cores.

Sharding: sequence-sharded layers (cores 0-3 batch 0, cores 4-7 batch 1;
core in-group index g owns row-blocks g and 7-g of its batch), one fused K/V
AllGather per layer within each 4-core group, a split (per row-block) 8-core
AllGather of the final LN output, then a vocab-sharded lm_head (each core
computes all 2048 rows x a 6283-wide vocab slice; host concatenates slices).

v2 scheduling changes vs v1:
- whole-tensor weight DMAs (wqkv 3 chunks, wo/w1 1 each, w2 4 chunks, whead 1)
  instead of per-m-tile loads: ~10 DMA issues/layer instead of ~120, and
  w2/wo are no longer fetched twice per layer.
- kf-outer FFN2 / kt-outer out_proj with 4 live PSUM accumulators.
- one shared 8-slot PSUM ring for all layer-phase accumulation/transposes.
- S psums merged into 512-wide banks: 3 exps + 3 mask-muls per head
  instead of 8 each.
- lm_head: whead preloaded in one DMA before the final AllGather; the final
  AllGather is split per row-block so the head starts on the first half's
  rows while the second gathers; logits written bf16 (host upcasts).

Compute: bf16 matmul inputs, fp32 PSUM/residual/LN. LN gammas are folded into
the following weights host-side, betas into biases. The causal mask is
multiplicative bf16 applied after exp (scores are small, so softmax's
max-subtraction is safely skipped); softmax denominators come from an
appended ones-column on V.
"""

from contextlib import ExitStack

import numpy as np
import ml_dtypes

import concourse.bass as bass
import concourse.mybir as mybir
import concourse.tile as tile
from concourse.bass_utils import run_bass_kernel_spmd
from concourse.vector_clock import ScopedClock, VectorClock

F32 = mybir.dt.float32
BF16 = mybir.dt.bfloat16
AF = mybir.ActivationFunctionType
BF = ml_dtypes.bfloat16
FP8 = mybir.dt.float8e4

V, E, N, H, L = 50257, 768, 1024, 12, 6
HD = E // H          # 64
FF = 4 * E           # 3072
B = 2
KT = E // 128         # 6 feature k-tiles
MT_QKV = 3 * KT       # 18 m-tiles for fused qkv
MT_FF = FF // 128     # 24
R = 256               # rows per core
NB = 8                # 128-row blocks per batch
VP = 6283             # vocab slice per core (8*6283 = 50264 >= V)
VPAD = 13 * 512       # host-side padded vocab slice (6656)
NCH = 13              # head vocab chunks of 512 (last used width = 139)
KV_GROUPS = [[0, 1, 2, 3], [4, 5, 6, 7]]
H8_GROUPS = [[0, 1, 2, 3, 4, 5, 6, 7]]
CC_KV_SZ = E * R + 2 * R * (H * 65)  # bytes: K as fp8, V as bf16
EPS = 1e-5


class _TileContext(tile.TileContext):
    """This image's walrus rejects Drain instructions with >1 sync-wait.
    Split the kernel-tail drain into one Drain per pending proc."""

    def _drain_and_barrier(self, tick_clock, wait_clock):
        nc = self.nc
        vec = tick_clock.global_clock
        n = len(vec)
        for proc in range(n):
            t = vec[proc]
            if t <= 0:
                continue
            sub = VectorClock([t if i == proc else 0 for i in range(n)])
            d = nc.sync.drain()
            wait_clock.add_sem_waits(d.ins, ScopedClock({None: sub}))
        nc.sync.drain()
        nc.all_engine_barrier()
        assert self.sems is not None
        popped = nc._tile_sem_poison_stack.pop()
        assert popped is self._sem_poison
        nc.clear_and_free_semaphores(list(self.sems.allocated().values()))
        nc.all_engine_barrier()


def _split_multi_waits(nc):
    """This walrus build encodes at most one sync-wait per instruction.
    Hoist extra waits onto NoOps inserted just before, on the same engine."""
    ctr = 0
    for bb in nc.main_func.blocks:
        il = bb.instructions
        out_l = []
        for ins in il:
            si = ins.sync_info
            if si is not None and si.on_wait is not None and len(si.on_wait) > 1:
                waits = list(si.on_wait)
                for w in waits[:-1]:
                    noop = mybir.InstNoOp(name=f"wsplit_{ctr}", ins=[], outs=[])
                    ctr += 1
                    noop.engine = ins.engine
                    noop.sync_info = type(si)(on_wait=[w], on_update=[])
                    out_l.append(noop)
                si.on_wait = waits[-1:]
            out_l.append(ins)
        il[:] = out_l


def _bcast_row(t, row, p=128):
    """AP reading DRAM row `t[row]` broadcast across p partitions."""
    base = t[row] if row is not None else t[:]
    return bass.AP(
        tensor=base.tensor, offset=base.offset,
        ap=[[0, p]] + [list(x) for x in base.ap])


def build_nc(use_bo, use_b2):
    nc = bass.Bass(num_devices=8)

    h0_in = nc.declare_dram_parameter("h0", [2, 128, E], F32, isOutput=False)
    # pre-tiled: [L, 18, 128, KT*128]  (m-tile, partition=feat%128, kt*128+mcol)
    wqkv_in = nc.declare_dram_parameter("wqkv", [L, MT_QKV, 128, E], BF16, isOutput=False)
    bqkv_in = nc.declare_dram_parameter("bqkv", [L, 3 * E], F32, isOutput=False)
    wo_in = nc.declare_dram_parameter("wo", [L, E, E], BF16, isOutput=False)
    w1_in = nc.declare_dram_parameter("w1", [L, MT_FF, 128, E], BF16, isOutput=False)
    b1_in = nc.declare_dram_parameter("b1", [L, FF], F32, isOutput=False)
    w2_in = nc.declare_dram_parameter("w2", [L, FF, E], BF16, isOutput=False)
    mask_in = nc.declare_dram_parameter("masks", [NB, 128, 256], BF16, isOutput=False)
    # pre-tiled: [NCH, 128, KT*512]
    wh_in = nc.declare_dram_parameter("whead", [NCH, 128, KT * 512], BF16, isOutput=False)
    ident_in = nc.declare_dram_parameter("ident", [128, 128], BF16, isOutput=False)
    bo_in = nc.declare_dram_parameter("bo", [L, E], F32, isOutput=False) if use_bo else None
    b2_in = nc.declare_dram_parameter("b2", [L, E], F32, isOutput=False) if use_b2 else None
    out = nc.declare_dram_parameter("logits", [B * N, VP], BF16, isOutput=True)

    cc_kv_in = [nc.dram_tensor(f"cckv_i{l}", [CC_KV_SZ], FP8) for l in range(L)]
    cc_kv_out = [nc.dram_tensor(f"cckv_o{l}", [4, CC_KV_SZ], FP8) for l in range(L)]
    cch_i = [nc.dram_tensor(f"cch_i{rb}", [E * 128], BF16) for rb in range(2)]
    cch_o = [nc.dram_tensor(f"cch_o{rb}", [8, E * 128], BF16, addr_space="Shared")
             for rb in range(2)]

    with _TileContext(nc) as tc, ExitStack() as ctx:
        const = ctx.enter_context(tc.tile_pool(name="const", bufs=1))

        ident = const.tile([128, 128], BF16)
        nc.sync.dma_start(out=ident, in_=ident_in[:])

        h_sb = [const.tile([128, E], F32, tag=f"h{rb}", name=f"h{rb}") for rb in range(2)]
        for rb in range(2):
            nc.sync.dma_start(out=h_sb[rb], in_=h0_in[rb])

        mask_sb = const.tile([128, NB, 256], BF16)

        hfT_all = const.tile([128, KT, B * N], BF16, tag="hfT_all")

        eps_t = const.tile([128, 1], F32)
        nc.vector.memset(eps_t, EPS)

        with ExitStack() as lctx:
            p = {
                "small": lctx.enter_context(tc.tile_pool(name="small", bufs=2)),
                "wgt": lctx.enter_context(tc.tile_pool(name="wgt", bufs=1)),
                "act": lctx.enter_context(tc.tile_pool(name="act", bufs=1)),
                "act2": lctx.enter_context(tc.tile_pool(name="act2", bufs=2)),
                "big": lctx.enter_context(tc.tile_pool(name="big", bufs=1)),
                "eSp": lctx.enter_context(tc.tile_pool(name="eSp", bufs=3)),
                "ps": lctx.enter_context(
                    tc.tile_pool(name="ps", bufs=8, space="PSUM")),
            }

            def psb(name, shape=(128, 512), dtype=F32):
                return p["ps"].tile(list(shape), dtype, tag="bank", name=name)

            def layernorm_to_T(xT_dst):
                for rb in range(2):
                    mv = p["small"].tile([128, nc.vector.BN_AGGR_DIM], F32, tag="ln_mv")
                    stats = p["small"].tile(
                        [128, 3, nc.vector.BN_STATS_DIM], F32, tag="ln_st")
                    xin = h_sb[rb]
                    for s in range(3):
                        nc.vector.bn_stats(
                            out=stats[:, s, :], in_=xin[:, s * 256:(s + 1) * 256])
                    nc.vector.bn_aggr(out=mv, in_=stats)
                    rstd = p["small"].tile([128, 1], F32, tag="ln_rstd")
                    nc.scalar.activation(
                        out=rstd, in_=mv[:, 1:2], func=AF.Sqrt, bias=eps_t, scale=1.0)
                    nc.vector.reciprocal(out=rstd, in_=rstd)
                    y = p["act2"].tile([128, E], BF16, tag="ln_y", bufs=1)
                    nc.vector.tensor_scalar(
                        out=y, in0=xin, scalar1=mv[:, 0:1], scalar2=rstd,
                        op0=mybir.AluOpType.subtract, op1=mybir.AluOpType.mult)
                    for kt in range(KT):
                        pst = psb("psT", (128, 128), BF16)
                        nc.tensor.transpose(pst, y[:, kt * 128:(kt + 1) * 128], ident)
                        nc.vector.tensor_copy(
                            out=xT_dst[:, kt, rb * 128:(rb + 1) * 128], in_=pst)

            for l in range(L):
                # ---- weight prefetch: K chunk, V chunk, Q chunk ----
                wqkv_sb = p["wgt"].tile([128, MT_QKV, E], BF16, tag="wqkv")
                for c0, c1 in ((KT, 2 * KT), (2 * KT, 3 * KT), (0, KT)):
                    nc.sync.dma_start(
                        out=wqkv_sb[:, c0:c1, :],
                        in_=wqkv_in[l, c0:c1].rearrange("m q c -> q m c"))
                bq = p["small"].tile([128, MT_QKV], F32, tag="bqkv")
                nc.sync.dma_start(
                    out=bq, in_=bqkv_in[l].rearrange("(m q) -> q m", q=128))
                if l == 0:
                    nc.sync.dma_start(
                        out=mask_sb, in_=mask_in.rearrange("k p c -> p k c"))

                # ---- LN1 -> xT ----
                xT = p["act"].tile([128, KT, R], BF16, tag="xT")
                layernorm_to_T(xT)

                kT_c = p["act"].tile([128, KT, R], FP8, tag="kT_c")
                va_c = p["act"].tile([128, 2, H * 65], BF16, tag="va_c")
                nc.vector.memset(
                    va_c.rearrange("q a (h o) -> q a h o", o=65)[:, :, :, 64:65], 1.0)
                qT = p["act"].tile([128, KT, R], BF16, tag="qT")

                def qkv_mtile(m, dst_ap):
                    ps = psb("mmQKV")
                    for kt in range(KT):
                        nc.tensor.matmul(
                            ps[:, 0:R], wqkv_sb[:, m, kt * 128:(kt + 1) * 128],
                            xT[:, kt, :], start=(kt == 0), stop=(kt == KT - 1))
                    nc.vector.tensor_scalar_add(
                        out=dst_ap, in0=ps[:, 0:R], scalar1=bq[:, m:m + 1])

                # K m-tiles first (feed the collective), then V, then ship
                for mk in range(KT):
                    qkv_mtile(KT + mk, kT_c[:, mk, :])
                for mv_ in range(KT):
                    vT_t = p["act2"].tile([128, R], BF16, tag="vT_t")
                    qkv_mtile(2 * KT + mv_, vT_t[:, :])
                    for rb in range(2):
                        pst = psb("psT", (128, 128), BF16)
                        nc.tensor.transpose(
                            pst, vT_t[:, rb * 128:(rb + 1) * 128], ident)
                        for hh in range(2):
                            h_abs = 2 * mv_ + hh
                            nc.vector.tensor_copy(
                                out=va_c[:, rb, h_abs * 65:h_abs * 65 + 64],
                                in_=pst[:, hh * 64:hh * 64 + 64])
                nc.sync.dma_start(
                    out=cc_kv_in[l][0:E * R].rearrange(
                        "(kt q c) -> q kt c", q=128, c=R),
                    in_=kT_c)
                nc.sync.dma_start(
                    out=cc_kv_in[l][E * R:].bitcast(BF16).rearrange(
                        "(a q c) -> q a c", q=128, a=2),
                    in_=va_c)
                nc.gpsimd.collective_compute(
                    "AllGather", mybir.AluOpType.bypass, replica_groups=KV_GROUPS,
                    ins=[cc_kv_in[l][:]], outs=[cc_kv_out[l][:]])

                # weight DMAs for the rest of the layer (issued before the
                # collective-gated pulls so they stream during attention)
                wo_sb = p["wgt"].tile([128, KT, E], BF16, tag="wo")
                nc.sync.dma_start(
                    out=wo_sb, in_=wo_in[l].rearrange("(kt q) c -> q kt c", q=128))
                w1_sb = p["wgt"].tile([128, MT_FF, E], BF16, tag="w1")
                nc.sync.dma_start(
                    out=w1_sb, in_=w1_in[l].rearrange("m q c -> q m c"))
                w2_sb = p["wgt"].tile([128, MT_FF, E], BF16, tag="w2")
                nc.sync.dma_start(
                    out=w2_sb, in_=w2_in[l].rearrange("(kf q) c -> q kf c", q=128))
                b1s = p["small"].tile([128, MT_FF], F32, tag="b1s")
                nc.sync.dma_start(
                    out=b1s, in_=b1_in[l].rearrange("(m q) -> q m", q=128))
                bo_b = None
                if bo_in is not None:
                    bo_b = p["small"].tile([128, E], F32, tag="bo_b")
                    nc.sync.dma_start(out=bo_b, in_=_bcast_row(bo_in, l))

                # Q m-tiles while the collective runs
                for mq in range(KT):
                    qkv_mtile(mq, qT[:, mq, :])

                # ---- pull gathered K/V ----
                kT_all = p["big"].tile([128, KT, N], FP8, tag="kT_all")
                V_all = p["big"].tile([128, NB, H * 65], BF16, tag="V_all")
                kview = cc_kv_out[l][:, 0:E * R].rearrange("g (f c) -> g f c", c=R)
                vview = cc_kv_out[l][:, E * R:].bitcast(BF16).rearrange(
                    "g (r x) -> g r x", x=H * 65)
                for g in range(4):
                    for half in range(2):
                        kb = g if half == 0 else 7 - g
                        nc.sync.dma_start(
                            out=kT_all[:, :, kb * 128:(kb + 1) * 128],
                            in_=kview[g].rearrange("(kt q) c -> q kt c", q=128)[
                                :, :, half * 128:(half + 1) * 128])
                        nc.sync.dma_start(
                            out=V_all[:, kb, :],
                            in_=vview[g, half * 128:(half + 1) * 128, :])

                # ---- attention (2-head software pipeline: S runs 2 heads
                # ahead of AV so AV never stalls on exp/mask) ----
                o_pack = p["act"].tile([128, 2, E], BF16, tag="o_pack")
                eS_of = {}

                def emit_S(hh):
                    base = (hh % 2) * 64
                    rhs_q = qT[base:base + 64, hh // 2, :]
                    eS = p["eSp"].tile([128, 4 * 256 + 4 * 128], BF16, tag="eS",
                                       name=f"eS_{l}_{hh}")
                    eS_of[hh] = eS
                    # kb pairs (0,1) and (2,3): 256-wide q, one 512 psum each
                    for pr in range(2):
                        ps = psb("mmS")
                        for j in range(2):
                            kb = 2 * pr + j
                            nc.tensor.matmul(
                                ps[:, j * 256:(j + 1) * 256],
                                kT_all[base:base + 64, hh // 2,
                                       kb * 128:(kb + 1) * 128],
                                rhs_q, start=True, stop=True)
                        nc.scalar.activation(
                            out=eS[:, pr * 512:(pr + 1) * 512], in_=ps, func=AF.Exp)
                        nc.vector.tensor_mul(
                            out=eS[:, pr * 512:(pr + 1) * 512],
                            in0=eS[:, pr * 512:(pr + 1) * 512],
                            in1=mask_sb[:, 2 * pr:2 * pr + 2, :])
                    # kb 4..7: 128-wide (second q block only), one 512 psum
                    ps = psb("mmS")
                    for j in range(4):
                        nc.tensor.matmul(
                            ps[:, j * 128:(j + 1) * 128],
                            kT_all[base:base + 64, hh // 2,
                                   (4 + j) * 128:(5 + j) * 128],
                            rhs_q[:, 128:256], start=True, stop=True)
                    nc.scalar.activation(
                        out=eS[:, 1024:1536], in_=ps, func=AF.Exp)
                    nc.vector.tensor_mul(
                        out=eS[:, 1024:1536], in0=eS[:, 1024:1536],
                        in1=mask_sb[:, 4:8, 128:256])

                def emit_AV(hh):
                    eS = eS_of.pop(hh)
                    for qb in range(2):
                        nkb = 4 if qb == 0 else NB
                        psO = psb("psO", (128, 65))
                        for kb in range(nkb):
                            if kb < 4:
                                sl = eS[:, kb * 256 + qb * 128:
                                        kb * 256 + qb * 128 + 128]
                            else:
                                sl = eS[:, 1024 + (kb - 4) * 128:
                                        1024 + (kb - 4) * 128 + 128]
                            nc.tensor.matmul(
                                psO, sl, V_all[:, kb, hh * 65:(hh + 1) * 65],
                                start=(kb == 0), stop=(kb == nkb - 1))
                        recip = p["small"].tile([128, 1], F32, tag="recip")
                        nc.vector.reciprocal(out=recip, in_=psO[:, 64:65])
                        nc.vector.tensor_scalar_mul(
                            out=o_pack[:, qb, hh * 64:(hh + 1) * 64],
                            in0=psO[:, 0:64], scalar1=recip)

                emit_S(0)
                emit_S(1)
                for hh in range(H):
                    if hh + 2 < H:
                        emit_S(hh + 2)
                    emit_AV(hh)

                oT = p["act"].tile([128, KT, R], BF16, tag="oT")
                for qb in range(2):
                    for f in range(KT):
                        pst = psb("psT", (128, 128), BF16)
                        nc.tensor.transpose(
                            pst, o_pack[:, qb, f * 128:(f + 1) * 128], ident)
                        nc.vector.tensor_copy(
                            out=oT[:, f, qb * 128:(qb + 1) * 128], in_=pst)

                # ---- out_proj + residual (kt-outer: wo read once) ----
                psr = [psb(f"psra_{l}_{i}", (128, 384)) for i in range(4)]
                for kt in range(KT):
                    for rb in range(2):
                        for half in range(2):
                            nc.tensor.matmul(
                                psr[2 * rb + half],
                                oT[:, kt, rb * 128:(rb + 1) * 128],
                                wo_sb[:, kt, half * 384:(half + 1) * 384],
                                start=(kt == 0), stop=(kt == KT - 1))
                for rb in range(2):
                    for half in range(2):
                        hs = h_sb[rb][:, half * 384:(half + 1) * 384]
                        nc.vector.tensor_add(out=hs, in0=hs, in1=psr[2 * rb + half])
                    if bo_b is not None:
                        nc.vector.tensor_add(out=h_sb[rb], in0=h_sb[rb], in1=bo_b)

                # ---- LN2 -> x2T ----
                x2T = p["act"].tile([128, KT, R], BF16, tag="x2T")
                layernorm_to_T(x2T)

                b2_b = None
                if b2_in is not None:
                    b2_b = p["small"].tile([128, E], F32, tag="b2_b")
                    nc.sync.dma_start(out=b2_b, in_=_bcast_row(b2_in, l))

                # ---- FFN1 (gelu+bias at evict) ----
                gT = p["big"].tile([128, MT_FF, R], BF16, tag="gT")
                for m in range(MT_FF):
                    ps = psb("mmF1")
                    for kt in range(KT):
                        nc.tensor.matmul(
                            ps[:, 0:R], w1_sb[:, m, kt * 128:(kt + 1) * 128],
                            x2T[:, kt, :], start=(kt == 0), stop=(kt == KT - 1))
                    nc.scalar.activation(
                        out=gT[:, m, :], in_=ps[:, 0:R], func=AF.Gelu_apprx_tanh,
                        bias=b1s[:, m:m + 1], scale=1.0)

                # ---- FFN2 + residual (kf-outer: w2 resident) ----
                psf = [psb(f"psrb_{l}_{i}", (128, 384)) for i in range(4)]
                for kf in range(MT_FF):
                    for rb in range(2):
                        for half in range(2):
                            nc.tensor.matmul(
                                psf[2 * rb + half],
                                gT[:, kf, rb * 128:(rb + 1) * 128],
                                w2_sb[:, kf, half * 384:(half + 1) * 384],
                                start=(kf == 0), stop=(kf == MT_FF - 1))
                for rb in range(2):
                    for half in range(2):
                        hs = h_sb[rb][:, half * 384:(half + 1) * 384]
                        nc.vector.tensor_add(out=hs, in0=hs, in1=psf[2 * rb + half])
                    if b2_b is not None:
                        nc.vector.tensor_add(out=h_sb[rb], in0=h_sb[rb], in1=b2_b)

        # ---- lm_head: preload whead, split final AllGather per row block ----
        with ExitStack() as hctx:
            whp = hctx.enter_context(tc.tile_pool(name="whp", bufs=1))
            hsm = hctx.enter_context(tc.tile_pool(name="hsm", bufs=2))
            lsb = hctx.enter_context(tc.tile_pool(name="lsb", bufs=6))
            psH = hctx.enter_context(tc.tile_pool(name="psH", bufs=8, space="PSUM"))

            def psh(name, shape=(128, 512), dtype=F32):
                return psH.tile(list(shape), dtype, tag="bank", name=name)

            wh_all = whp.tile([128, NCH, KT * 512], BF16, tag="wh")
            nc.sync.dma_start(out=wh_all, in_=wh_in.rearrange("n q c -> q n c"))

            # final LN per row block -> ship -> AllGather (split per rb)
            hfT = hsm.tile([128, KT, R], BF16, tag="hfT", bufs=1)
            for rb in range(2):
                mv = hsm.tile([128, nc.vector.BN_AGGR_DIM], F32, tag="ln_mv")
                stats = hsm.tile([128, 3, nc.vector.BN_STATS_DIM], F32, tag="ln_st")
                for s in range(3):
                    nc.vector.bn_stats(
                        out=stats[:, s, :], in_=h_sb[rb][:, s * 256:(s + 1) * 256])
                nc.vector.bn_aggr(out=mv, in_=stats)
                rstd = hsm.tile([128, 1], F32, tag="ln_rstd")
                nc.scalar.activation(
                    out=rstd, in_=mv[:, 1:2], func=AF.Sqrt, bias=eps_t, scale=1.0)
                nc.vector.reciprocal(out=rstd, in_=rstd)
                y = hsm.tile([128, E], BF16, tag="ln_y")
                nc.vector.tensor_scalar(
                    out=y, in0=h_sb[rb], scalar1=mv[:, 0:1], scalar2=rstd,
                    op0=mybir.AluOpType.subtract, op1=mybir.AluOpType.mult)
                for kt in range(KT):
                    pst = psh("psTh", (128, 128), BF16)
                    nc.tensor.transpose(pst, y[:, kt * 128:(kt + 1) * 128], ident)
                    nc.vector.tensor_copy(
                        out=hfT[:, kt, rb * 128:(rb + 1) * 128], in_=pst)
                nc.sync.dma_start(
                    out=cch_i[rb][:].rearrange("(kt q c) -> q kt c", q=128, c=128),
                    in_=hfT[:, :, rb * 128:(rb + 1) * 128])
                nc.gpsimd.collective_compute(
                    "AllGather", mybir.AluOpType.bypass, replica_groups=H8_GROUPS,
                    ins=[cch_i[rb][:]], outs=[cch_o[rb][:]])

            # pulls: rb0 gives global blocks (bb*8+g), rb1 gives (bb*8+7-g)
            rt_of = {0: [], 1: []}
            for rb in range(2):
                hgv = cch_o[rb].rearrange("g (f c) -> g f c", c=128)
                for src in range(8):
                    bb, g = src // 4, src % 4
                    rt = bb * NB + (g if rb == 0 else 7 - g)
                    rt_of[rb].append(rt)
                    nc.sync.dma_start(
                        out=hfT_all[:, :, rt * 128:(rt + 1) * 128],
                        in_=hgv[src].rearrange("(kt q) c -> q kt c", q=128))

            # head: first the rb0 rows (available first), then rb1 rows
            for i, rt in enumerate(rt_of[0] + rt_of[1]):
                for nch in range(NCH):
                    wN = 512 if nch < NCH - 1 else VP - 512 * (NCH - 1)
                    ps = psh("psHmm")
                    for kt in range(KT):
                        nc.tensor.matmul(
                            ps[:, 0:wN], hfT_all[:, kt, rt * 128:(rt + 1) * 128],
                            wh_all[:, nch, kt * 512:kt * 512 + wN],
                            start=(kt == 0), stop=(kt == KT - 1))
                    ls = lsb.tile([128, 512], BF16, tag="ls")
                    if nch % 2 == 0:
                        nc.vector.tensor_copy(out=ls[:, 0:wN], in_=ps[:, 0:wN])
                    else:
                        nc.scalar.activation(
                            out=ls[:, 0:wN], in_=ps[:, 0:wN], func=AF.Copy)
                    nc.sync.dma_start(
                        out=out[rt * 128:(rt + 1) * 128, nch * 512:nch * 512 + wN],
                        in_=ls[:, 0:wN])
    _split_multi_waits(nc)
    return nc


# ---------------------------------------------------------------------------
# host side
# ---------------------------------------------------------------------------

def _sinusoidal_pos(n, dim):
    pos = np.arange(n, dtype=np.float32)[:, None]
    i = np.arange(0, dim, 2, dtype=np.float32)
    j = np.arange(1, dim, 2, dtype=np.float32)
    s = np.sin(pos / np.power(np.float32(10000.0), 2.0 * i / dim, dtype=np.float32))
    c = np.cos(pos / np.power(np.float32(10000.0), 2.0 * j / dim, dtype=np.float32))
    return np.stack([s, c], axis=-1).reshape(n, dim).astype(np.float32)


_CACHE = {}


def _get_nc(use_bo, use_b2):
    key = (use_bo, use_b2)
    if key not in _CACHE:
        _CACHE[key] = build_nc(use_bo, use_b2)
    return _CACHE[key]


def _tile_w(w):
    """[E, M*128] -> [M, 128, KT*128]: [m, p, kt*128+c] = w[kt*128+p, m*128+c]."""
    M = w.shape[1] // 128
    return np.ascontiguousarray(
        w.reshape(KT, 128, M, 128).transpose(2, 1, 0, 3).reshape(M, 128, KT * 128))


def kernel(x, tok_emb, wq, wk, wv, wo, bo, ln1_g, ln1_b, ln2_g, ln2_b,
           w1, b1, w2, b2, lnf_g, lnf_b, w_head, _trace=False):
    x = np.asarray(x)
    f = lambda a: np.asarray(a, dtype=np.float32)
    tok_emb, wq, wk, wv, wo = f(tok_emb), f(wq), f(wk), f(wv), f(wo)
    bo, w1, b1, w2, b2 = f(bo), f(w1), f(b1), f(w2), f(b2)
    ln1_g, ln1_b, ln2_g, ln2_b = f(ln1_g), f(ln1_b), f(ln2_g), f(ln2_b)
    lnf_g, lnf_b, w_head = f(lnf_g), f(lnf_b), f(w_head)

    h0 = tok_emb[x] + _sinusoidal_pos(N, E)[None, :, :]     # [B, N, E] f32

    scale = np.float32(1.0 / np.sqrt(HD))
    wqkv = np.concatenate([wq * scale, wk, wv], axis=2)      # [L, E, 3E]
    bqkv = np.einsum("le,lef->lf", ln1_b, wqkv).astype(np.float32)
    wqkv = (ln1_g[:, :, None] * wqkv).astype(BF)
    wqkv_t = np.stack([_tile_w(wqkv[l]) for l in range(L)])
    b1c = (b1 + np.einsum("le,lef->lf", ln2_b, w1)).astype(np.float32)
    w1f = (ln2_g[:, :, None] * w1).astype(BF)
    w1_t = np.stack([_tile_w(w1f[l]) for l in range(L)])
    w2f = np.ascontiguousarray(w2.astype(BF))
    wof = np.ascontiguousarray(wo.astype(BF))
    whf = np.zeros((E, 8 * VPAD), dtype=np.float32)
    wh_scaled = lnf_g[:, None] * w_head
    for c in range(8):
        lo, hi = c * VP, min((c + 1) * VP, V)
        whf[:, c * VPAD:c * VPAD + (hi - lo)] = wh_scaled[:, lo:hi]
    whf = whf.astype(BF)

    use_bo = bool(np.any(bo))
    use_b2 = bool(np.any(b2))
    nc = _get_nc(use_bo, use_b2)

    ident = np.eye(128, dtype=BF)
    key_idx = np.arange(N)[:, None]
    in_maps = []
    for c in range(8):
        bb, g = c // 4, c % 4
        blocks = [g, 7 - g]
        h0c = np.stack([h0[bb, blk * 128:(blk + 1) * 128, :] for blk in blocks])
        masks = np.zeros((NB, 128, 256), dtype=BF)
        for qi, blk in enumerate(blocks):
            q = blk * 128 + np.arange(128)[None, :]
            allow = (key_idx <= q).astype(np.float32).reshape(NB, 128, 128)
            masks[:, :, qi * 128:(qi + 1) * 128] = allow.astype(BF)
        # whead slice, re-tiled to [NCH, 128, KT*512]
        whc = whf[:, c * VPAD:(c + 1) * VPAD]
        whc_t = np.ascontiguousarray(
            whc.reshape(KT, 128, NCH, 512).transpose(2, 1, 0, 3).reshape(
                NCH, 128, KT * 512))
        m = {
            "h0": np.ascontiguousarray(h0c, dtype=np.float32),
            "wqkv": wqkv_t, "bqkv": bqkv, "wo": wof,
            "w1": w1_t, "b1": b1c, "w2": w2f,
            "masks": masks, "whead": whc_t, "ident": ident,
        }
        if use_bo:
            m["bo"] = bo
        if use_b2:
            m["b2"] = b2
        in_maps.append(m)

    res = run_bass_kernel_spmd(nc, in_maps, list(range(8)), trace=_trace)
    logits = np.concatenate(
        [res.results[c]["logits"].astype(np.float32) for c in range(8)], axis=1)
    logits = logits[:, :V]
    if np.any(lnf_b):
        logits = logits + (lnf_b @ w_head)[None, :]
    out = logits.reshape(B, N, V)
    if _trace:
        return out, res
    return out
